# revision 5
# baseline (speedup 1.0000x reference)
"""Trainium2 Bass kernel for nn_CategoryAdder (embedding lookup + masked add).

Computation: out[b,s,:] = inputs[b,s,:] + emb where
  emb = table[categories[b,s]] masked to zero when categories[b,s]==0 or
  s == mask_positions[b].

Host-side preprocessing folds both masks into the data:
  - categories[b, mask_positions[b]] = 0
  - table row 0 zeroed (on a copy)
so the device computes exactly: out = inputs + table0[categories].

Sharding: data-parallel over batch across 8 NeuronCores (8 batches per core,
16384 tokens/core). Table replicated. Per core the kernel loops over tiles of
T tokens: HWDGE-load the input tile, SWDGE dma_gather the table rows
(2 KB each) from HBM by precomputed int16 indices, DVE add, HWDGE-store.
"""

import numpy as np

import concourse.bass as bass
import concourse.mybir as mybir
from concourse import bacc, tile
from concourse.bass_utils import run_bass_kernel_spmd

B, S, D = 64, 2048, 512
N_CAT = 5000
N_CORES = 8
B_PER = B // N_CORES          # 8 batches per core
NTOK = B_PER * S              # 16384 tokens per core
T = 1024                      # tokens per tile (dma_gather single_packet caps at 1024 idxs)
NT = NTOK // T                # tiles per core
C = T // 128                  # column blocks per tile (tokens per partition)
IDX_COLS = NTOK // 16         # columns of the wrapped int16 index tensor

# dma_gather writes gather-slot i to SBUF (partition i%128, column i//128).
# Our input/output tiles place token t at (partition t//C, column t%C), so
# gather slot i must hold the category of token (i%128)*C + i//128.
_slot_to_token = (np.arange(T) % 128) * C + (np.arange(T) // 128)


def _build_nc():
    nc = bacc.Bacc("TRN2", target_bir_lowering=False, debug=False)
    x = nc.dram_tensor("x", [NTOK, D], mybir.dt.float32, kind="ExternalInput")
    tbl = nc.dram_tensor("tbl", [N_CAT, D], mybir.dt.float32, kind="ExternalInput")
    idx = nc.dram_tensor("idx", [128, IDX_COLS], mybir.dt.int16, kind="ExternalInput")
    out = nc.dram_tensor("out", [NTOK, D], mybir.dt.float32, kind="ExternalOutput")

    with tile.TileContext(nc) as tc:
        with (
            tc.tile_pool(name="idxp", bufs=1) as idxp,
            tc.tile_pool(name="inp", bufs=3) as inp,
            tc.tile_pool(name="embp", bufs=3) as embp,
        ):
            idx_sb = idxp.tile([128, IDX_COLS], mybir.dt.int16)
            nc.sync.dma_start(out=idx_sb[:], in_=idx[:, :])
            for j in range(NT):
                in_t = inp.tile([128, C * D], mybir.dt.float32)
                nc.sync.dma_start(
                    out=in_t[:],
                    in_=x[j * T : (j + 1) * T].rearrange("(p c) e -> p (c e)", p=128),
                )
                emb_t = embp.tile([128, C * D], mybir.dt.float32)
                nc.gpsimd.dma_gather(
                    emb_t[:].rearrange("p (c e) -> p c e", e=D),
                    tbl[:, :],
                    idx_sb[:, j * (T // 16) : (j + 1) * (T // 16)],
                    T,
                    T,
                    D,
                )
                nc.vector.tensor_add(out=in_t[:], in0=in_t[:], in1=emb_t[:])
                nc.sync.dma_start(
                    out=out[j * T : (j + 1) * T].rearrange("(p c) e -> p (c e)", p=128),
                    in_=in_t[:],
                )
    nc.compile()
    return nc


def _prep_idx(cat_shard: np.ndarray) -> np.ndarray:
    """cat_shard: (NTOK,) int -> wrapped int16 index tensor [128, IDX_COLS].

    Per tile of T tokens: permute by gather slot, then wrap 16-way
    (idxs[p, s] = slot s*16+p) and replicate across the 8 groups of 16
    partitions as the HW expects.
    """
    tiles = []
    for j in range(NT):
        vals = cat_shard[j * T : (j + 1) * T][_slot_to_token]
        wrapped = vals.reshape(T // 16, 16).T  # [16, T//16]
        tiles.append(np.tile(wrapped, (8, 1)))  # [128, T//16]
    return np.ascontiguousarray(np.concatenate(tiles, axis=1).astype(np.int16))


RUN_KWARGS = {}  # test harness can set e.g. {"trace": True}
LAST_RESULTS = None


def kernel(inputs, categories, mask_positions, table):
    global LAST_RESULTS
    inputs = np.asarray(inputs, dtype=np.float32)
    categories = np.asarray(categories).astype(np.int64)
    mask_positions = np.asarray(mask_positions).astype(np.int64)
    table = np.asarray(table, dtype=np.float32)

    # Fold both masks into the data.
    cat = categories.copy()
    cat[np.arange(B), mask_positions[:, 0]] = 0
    tbl0 = table.copy()
    tbl0[0] = 0.0

    nc = _build_nc()

    in_maps = []
    for c in range(N_CORES):
        x_shard = np.ascontiguousarray(
            inputs[c * B_PER : (c + 1) * B_PER].reshape(NTOK, D)
        )
        cat_shard = cat[c * B_PER : (c + 1) * B_PER].reshape(NTOK)
        in_maps.append({"x": x_shard, "tbl": tbl0, "idx": _prep_idx(cat_shard)})

    res = run_bass_kernel_spmd(
        nc, in_maps, core_ids=list(range(N_CORES)), **RUN_KWARGS
    )
    LAST_RESULTS = res
    out = np.concatenate(
        [r["out"].reshape(B_PER, S, D) for r in res.results], axis=0
    )
    return out


# revision 6
# speedup vs baseline: 1.1411x; 1.1411x over previous
"""Trainium2 Bass kernel for nn_CategoryAdder (embedding lookup + masked add).

Computation: out[b,s,:] = inputs[b,s,:] + emb where
  emb = table[categories[b,s]] masked to zero when categories[b,s]==0 or
  s == mask_positions[b].

Host-side preprocessing folds both masks into the data:
  - categories[b, mask_positions[b]] = 0
  - table row 0 zeroed (on a copy)
so the device computes exactly: out = inputs + table0[categories].

Sharding: data-parallel over batch across 8 NeuronCores (8 batches per core,
16384 tokens/core). Table replicated. Per core the kernel loops over tiles of
T tokens: HWDGE-load the input tile, SWDGE dma_gather the table rows
(2 KB each) from HBM by precomputed int16 indices, DVE add, HWDGE-store.
"""

import numpy as np

import concourse.bass as bass
import concourse.mybir as mybir
from concourse import bacc, tile
from concourse.bass_utils import run_bass_kernel_spmd

B, S, D = 64, 2048, 512
N_CAT = 5000
N_CORES = 8
B_PER = B // N_CORES          # 8 batches per core
NTOK = B_PER * S              # 16384 tokens per core
T = 1024                      # tokens per tile (dma_gather single_packet caps at 1024 idxs)
NT = NTOK // T                # tiles per core
C = T // 128                  # column blocks per tile (tokens per partition)
IDX_COLS = NTOK // 16         # columns of the wrapped int16 index tensor

# dma_gather writes gather-slot i to SBUF (partition i%128, column i//128).
# Our input/output tiles place token t at (partition t//C, column t%C), so
# gather slot i must hold the category of token (i%128)*C + i//128.
_slot_to_token = (np.arange(T) % 128) * C + (np.arange(T) // 128)


def _build_nc():
    nc = bacc.Bacc("TRN2", target_bir_lowering=False, debug=False)
    x = nc.dram_tensor("x", [NTOK, D], mybir.dt.float32, kind="ExternalInput")
    tbl = nc.dram_tensor("tbl", [N_CAT, D], mybir.dt.float32, kind="ExternalInput")
    idx = nc.dram_tensor("idx", [128, IDX_COLS], mybir.dt.int16, kind="ExternalInput")
    out = nc.dram_tensor("out", [NTOK, D], mybir.dt.float32, kind="ExternalOutput")

    with tile.TileContext(nc) as tc:
        with (
            tc.tile_pool(name="idxp", bufs=1) as idxp,
            tc.tile_pool(name="inp", bufs=3) as inp,
            tc.tile_pool(name="embp", bufs=3) as embp,
        ):
            idx_sb = idxp.tile([128, IDX_COLS], mybir.dt.int16)
            nc.sync.dma_start(out=idx_sb[:], in_=idx[:, :])
            for j in range(NT):
                in_t = inp.tile([128, C * D], mybir.dt.float32)
                nc.sync.dma_start(
                    out=in_t[:],
                    in_=x[j * T : (j + 1) * T].rearrange("(p c) e -> p (c e)", p=128),
                )
                emb_t = embp.tile([128, C * D], mybir.dt.float32)
                nc.gpsimd.dma_gather(
                    emb_t[:].rearrange("p (c e) -> p c e", e=D),
                    tbl[:, :],
                    idx_sb[:, j * (T // 16) : (j + 1) * (T // 16)],
                    T,
                    T,
                    D,
                    # multi-packet lets the SDMA engines start draining while
                    # Q7 is still generating descriptors (~9us per 1024 idxs);
                    # single_packet serializes gen->transfer and costs ~40us.
                    single_packet=False,
                )
                nc.vector.tensor_add(out=in_t[:], in0=in_t[:], in1=emb_t[:])
                nc.sync.dma_start(
                    out=out[j * T : (j + 1) * T].rearrange("(p c) e -> p (c e)", p=128),
                    in_=in_t[:],
                )
    nc.compile()
    return nc


def _prep_idx(cat_shard: np.ndarray) -> np.ndarray:
    """cat_shard: (NTOK,) int -> wrapped int16 index tensor [128, IDX_COLS].

    Per tile of T tokens: permute by gather slot, then wrap 16-way
    (idxs[p, s] = slot s*16+p) and replicate across the 8 groups of 16
    partitions as the HW expects.
    """
    tiles = []
    for j in range(NT):
        vals = cat_shard[j * T : (j + 1) * T][_slot_to_token]
        wrapped = vals.reshape(T // 16, 16).T  # [16, T//16]
        tiles.append(np.tile(wrapped, (8, 1)))  # [128, T//16]
    return np.ascontiguousarray(np.concatenate(tiles, axis=1).astype(np.int16))


RUN_KWARGS = {}  # test harness can set e.g. {"trace": True}
LAST_RESULTS = None


def kernel(inputs, categories, mask_positions, table):
    global LAST_RESULTS
    inputs = np.asarray(inputs, dtype=np.float32)
    categories = np.asarray(categories).astype(np.int64)
    mask_positions = np.asarray(mask_positions).astype(np.int64)
    table = np.asarray(table, dtype=np.float32)

    # Fold both masks into the data.
    cat = categories.copy()
    cat[np.arange(B), mask_positions[:, 0]] = 0
    tbl0 = table.copy()
    tbl0[0] = 0.0

    nc = _build_nc()

    in_maps = []
    for c in range(N_CORES):
        x_shard = np.ascontiguousarray(
            inputs[c * B_PER : (c + 1) * B_PER].reshape(NTOK, D)
        )
        cat_shard = cat[c * B_PER : (c + 1) * B_PER].reshape(NTOK)
        in_maps.append({"x": x_shard, "tbl": tbl0, "idx": _prep_idx(cat_shard)})

    res = run_bass_kernel_spmd(
        nc, in_maps, core_ids=list(range(N_CORES)), **RUN_KWARGS
    )
    LAST_RESULTS = res
    out = np.concatenate(
        [r["out"].reshape(B_PER, S, D) for r in res.results], axis=0
    )
    return out


# revision 7
# speedup vs baseline: 1.1516x; 1.0092x over previous
"""Trainium2 Bass kernel for nn_CategoryAdder (embedding lookup + masked add).

Computation: out[b,s,:] = inputs[b,s,:] + emb where
  emb = table[categories[b,s]] masked to zero when categories[b,s]==0 or
  s == mask_positions[b].

Host-side preprocessing folds both masks into the data:
  - categories[b, mask_positions[b]] = 0
  - table row 0 zeroed (on a copy)
so the device computes exactly: out = inputs + table0[categories].

Sharding: data-parallel over batch across 8 NeuronCores (8 batches per core,
16384 tokens/core). Table replicated. Per core the kernel loops over tiles of
T tokens: HWDGE-load the input tile, SWDGE dma_gather the table rows
(2 KB each) from HBM by precomputed int16 indices, DVE add, HWDGE-store.
"""

import numpy as np

import concourse.bass as bass
import concourse.mybir as mybir
from concourse import bacc, tile
from concourse.bass_utils import run_bass_kernel_spmd

B, S, D = 64, 2048, 512
N_CAT = 5000
N_CORES = 8
B_PER = B // N_CORES          # 8 batches per core
NTOK = B_PER * S              # 16384 tokens per core
T = 1024                      # tokens per tile (dma_gather single_packet caps at 1024 idxs)
NT = NTOK // T                # tiles per core
C = T // 128                  # column blocks per tile (tokens per partition)
IDX_COLS = NTOK // 16         # columns of the wrapped int16 index tensor

# dma_gather writes gather-slot i to SBUF (partition i%128, column i//128).
# Our input/output tiles place token t at (partition t//C, column t%C), so
# gather slot i must hold the category of token (i%128)*C + i//128.
_slot_to_token = (np.arange(T) % 128) * C + (np.arange(T) // 128)


def _build_nc():
    nc = bacc.Bacc("TRN2", target_bir_lowering=False, debug=False)
    x = nc.dram_tensor("x", [NTOK, D], mybir.dt.float32, kind="ExternalInput")
    tbl = nc.dram_tensor("tbl", [N_CAT, D], mybir.dt.float32, kind="ExternalInput")
    idx = nc.dram_tensor("idx", [128, IDX_COLS], mybir.dt.int16, kind="ExternalInput")
    out = nc.dram_tensor("out", [NTOK, D], mybir.dt.float32, kind="ExternalOutput")

    with tile.TileContext(nc) as tc:
        with (
            tc.tile_pool(name="idxp", bufs=1) as idxp,
            tc.tile_pool(name="inp", bufs=3) as inp,
            tc.tile_pool(name="embp", bufs=3) as embp,
        ):
            idx_sb = idxp.tile([128, IDX_COLS], mybir.dt.int16)
            nc.sync.dma_start(out=idx_sb[:], in_=idx[:, :])
            for j in range(NT):
                emb_t = embp.tile([128, C * D], mybir.dt.float32)
                nc.gpsimd.dma_gather(
                    emb_t[:].rearrange("p (c e) -> p c e", e=D),
                    tbl[:, :],
                    idx_sb[:, j * (T // 16) : (j + 1) * (T // 16)],
                    T,
                    T,
                    D,
                    # multi-packet lets the SDMA engines start draining while
                    # Q7 is still generating descriptors (~9us per 1024 idxs);
                    # single_packet serializes gen->transfer and costs ~40us.
                    single_packet=False,
                )
                in_t = inp.tile([128, C * D], mybir.dt.float32)
                nc.sync.dma_start(
                    out=in_t[:],
                    in_=x[j * T : (j + 1) * T].rearrange("(p c) e -> p (c e)", p=128),
                )
                nc.vector.tensor_add(out=in_t[:], in0=in_t[:], in1=emb_t[:])
                nc.sync.dma_start(
                    out=out[j * T : (j + 1) * T].rearrange("(p c) e -> p (c e)", p=128),
                    in_=in_t[:],
                )
    nc.compile()
    return nc


def _prep_idx(cat_shard: np.ndarray) -> np.ndarray:
    """cat_shard: (NTOK,) int -> wrapped int16 index tensor [128, IDX_COLS].

    Per tile of T tokens: permute by gather slot, then wrap 16-way
    (idxs[p, s] = slot s*16+p) and replicate across the 8 groups of 16
    partitions as the HW expects.
    """
    tiles = []
    for j in range(NT):
        vals = cat_shard[j * T : (j + 1) * T][_slot_to_token]
        wrapped = vals.reshape(T // 16, 16).T  # [16, T//16]
        tiles.append(np.tile(wrapped, (8, 1)))  # [128, T//16]
    return np.ascontiguousarray(np.concatenate(tiles, axis=1).astype(np.int16))


RUN_KWARGS = {}  # test harness can set e.g. {"trace": True}
LAST_RESULTS = None


def kernel(inputs, categories, mask_positions, table):
    global LAST_RESULTS
    inputs = np.asarray(inputs, dtype=np.float32)
    categories = np.asarray(categories).astype(np.int64)
    mask_positions = np.asarray(mask_positions).astype(np.int64)
    table = np.asarray(table, dtype=np.float32)

    # Fold both masks into the data.
    cat = categories.copy()
    cat[np.arange(B), mask_positions[:, 0]] = 0
    tbl0 = table.copy()
    tbl0[0] = 0.0

    nc = _build_nc()

    in_maps = []
    for c in range(N_CORES):
        x_shard = np.ascontiguousarray(
            inputs[c * B_PER : (c + 1) * B_PER].reshape(NTOK, D)
        )
        cat_shard = cat[c * B_PER : (c + 1) * B_PER].reshape(NTOK)
        in_maps.append({"x": x_shard, "tbl": tbl0, "idx": _prep_idx(cat_shard)})

    res = run_bass_kernel_spmd(
        nc, in_maps, core_ids=list(range(N_CORES)), **RUN_KWARGS
    )
    LAST_RESULTS = res
    out = np.concatenate(
        [r["out"].reshape(B_PER, S, D) for r in res.results], axis=0
    )
    return out


# revision 9
# speedup vs baseline: 1.1540x; 1.0021x over previous
"""Trainium2 Bass kernel for nn_CategoryAdder (embedding lookup + masked add).

Computation: out[b,s,:] = inputs[b,s,:] + emb where
  emb = table[categories[b,s]] masked to zero when categories[b,s]==0 or
  s == mask_positions[b].

Host-side preprocessing folds both masks into the data:
  - categories[b, mask_positions[b]] = 0
  - table row 0 zeroed (on a copy)
so the device computes exactly: out = inputs + table0[categories].

Sharding: data-parallel over batch across 8 NeuronCores (8 batches per core,
16384 tokens/core). Table replicated. Per core the kernel loops over tiles of
T tokens: HWDGE-load the input tile, SWDGE dma_gather the table rows
(2 KB each) from HBM by precomputed int16 indices, DVE add, HWDGE-store.
"""

import numpy as np

import concourse.bass as bass
import concourse.mybir as mybir
from concourse import bacc, tile
from concourse.bass_utils import run_bass_kernel_spmd

B, S, D = 64, 2048, 512
N_CAT = 5000
N_CORES = 8
B_PER = B // N_CORES          # 8 batches per core
NTOK = B_PER * S              # 16384 tokens per core
T = 1024                      # tokens per tile (dma_gather single_packet caps at 1024 idxs)
NT = NTOK // T                # tiles per core
C = T // 128                  # column blocks per tile (tokens per partition)
IDX_COLS = NTOK // 16         # columns of the wrapped int16 index tensor

# dma_gather writes gather-slot i to SBUF (partition i%128, column i//128).
# Our input/output tiles place token t at (partition t//C, column t%C), so
# gather slot i must hold the category of token (i%128)*C + i//128.
_slot_to_token = (np.arange(T) % 128) * C + (np.arange(T) // 128)


def _build_nc():
    nc = bacc.Bacc("TRN2", target_bir_lowering=False, debug=False)
    x = nc.dram_tensor("x", [NTOK, D], mybir.dt.float32, kind="ExternalInput")
    tbl = nc.dram_tensor("tbl", [N_CAT, D], mybir.dt.float32, kind="ExternalInput")
    idx = nc.dram_tensor("idx", [128, IDX_COLS], mybir.dt.int16, kind="ExternalInput")
    out = nc.dram_tensor("out", [NTOK, D], mybir.dt.float32, kind="ExternalOutput")

    with tile.TileContext(nc) as tc:
        with (
            tc.tile_pool(name="idxp", bufs=1) as idxp,
            tc.tile_pool(name="inp", bufs=3) as inp,
            tc.tile_pool(name="embp", bufs=3) as embp,
        ):
            idx_sb = idxp.tile([128, IDX_COLS], mybir.dt.int16)
            nc.sync.dma_start(out=idx_sb[:], in_=idx[:, :])
            for j in range(NT):
                emb_t = embp.tile([128, C * D], mybir.dt.float32)
                nc.gpsimd.dma_gather(
                    emb_t[:].rearrange("p (c e) -> p c e", e=D),
                    tbl[:, :],
                    idx_sb[:, j * (T // 16) : (j + 1) * (T // 16)],
                    T,
                    T,
                    D,
                    # multi-packet lets the SDMA engines start draining while
                    # Q7 is still generating descriptors (~9us per 1024 idxs);
                    # single_packet serializes gen->transfer and costs ~40us.
                    single_packet=False,
                )
                in_t = inp.tile([128, C * D], mybir.dt.float32)
                nc.sync.dma_start(
                    out=in_t[:],
                    in_=x[j * T : (j + 1) * T].rearrange("(p c) e -> p (c e)", p=128),
                )
                nc.vector.tensor_add(out=in_t[:], in0=in_t[:], in1=emb_t[:])
                nc.sync.dma_start(
                    out=out[j * T : (j + 1) * T].rearrange("(p c) e -> p (c e)", p=128),
                    in_=in_t[:],
                )
    nc.compile()
    return nc


def _prep_idx(cat_shard: np.ndarray) -> np.ndarray:
    """cat_shard: (NTOK,) int -> wrapped int16 index tensor [128, IDX_COLS].

    Per tile of T tokens: permute by gather slot, then wrap 16-way
    (idxs[p, s] = slot s*16+p) and replicate across the 8 groups of 16
    partitions as the HW expects.
    """
    tiles = []
    for j in range(NT):
        vals = cat_shard[j * T : (j + 1) * T][_slot_to_token]
        wrapped = vals.reshape(T // 16, 16).T  # [16, T//16]
        tiles.append(np.tile(wrapped, (8, 1)))  # [128, T//16]
    return np.ascontiguousarray(np.concatenate(tiles, axis=1).astype(np.int16))


RUN_KWARGS = {}  # test harness can set e.g. {"trace": True}
LAST_RESULTS = None
_NC = None


def _get_nc():
    global _NC
    if _NC is None:
        _NC = _build_nc()
    return _NC


def kernel(inputs, categories, mask_positions, table):
    global LAST_RESULTS
    inputs = np.asarray(inputs, dtype=np.float32)
    categories = np.asarray(categories).astype(np.int64)
    mask_positions = np.asarray(mask_positions).astype(np.int64)
    table = np.asarray(table, dtype=np.float32)

    # Fold both masks into the data.
    cat = categories.copy()
    cat[np.arange(B), mask_positions[:, 0]] = 0
    tbl0 = table.copy()
    tbl0[0] = 0.0

    nc = _get_nc()

    in_maps = []
    for c in range(N_CORES):
        x_shard = np.ascontiguousarray(
            inputs[c * B_PER : (c + 1) * B_PER].reshape(NTOK, D)
        )
        cat_shard = cat[c * B_PER : (c + 1) * B_PER].reshape(NTOK)
        in_maps.append({"x": x_shard, "tbl": tbl0, "idx": _prep_idx(cat_shard)})

    res = run_bass_kernel_spmd(
        nc, in_maps, core_ids=list(range(N_CORES)), **RUN_KWARGS
    )
    LAST_RESULTS = res
    out = np.concatenate(
        [r["out"].reshape(B_PER, S, D) for r in res.results], axis=0
    )
    return out
